# revision 1
# baseline (speedup 1.0000x reference)
"""Trainium2 Bass kernel for fused causal multi-head attention
(qkv projection + causal softmax attention), B=2, T=4096, C=768, nH=12.

Sharding: 8 cores, core c -> batch b=c//4, head group g=c%4 (3 heads each).
Each core computes qkv projection for its 3 heads from x[b] (host-transposed
to xT [C,T]) with a column-sharded, column-reordered weight stack, then
attention in the S^T orientation:
  S^T[k,q] = K Q^T  (lhsT = K^T slices, rhs = Q^T)  -> psum
  P^T = exp(S^T/8)  (scalar engine, psum -> sbuf fp32r)
  O^T = [V|1]^T P^T (ones column gives the softmax denominator)
  O   = transpose(O^T) / denom   -> DMA out
All matmuls run in float32r (TF32-class, 4x the fp32 rate).
"""
import sys
sys.path.insert(0, '/opt/trn_rl_repo')
import numpy as np

import concourse.bass as bass
import concourse.tile as tile
from concourse import bacc, mybir
from concourse import bass_utils

B, T, C, NH = 2, 4096, 768, 12
HD = 64
HPC = 3            # heads per core
NCORES = 8
NQ = T // 512      # q-chunks of 512
NKC = T // 128     # k-chunks of 128
GRP = 3            # k-chunks per exp group (psum [128, 1536] x2 bufs)
NEG = -1.0e35

FR = mybir.dt.float32r
F32 = mybir.dt.float32
AF = mybir.ActivationFunctionType
AL = mybir.AluOpType

_CACHE = {}


def _build():
    if 'nc' in _CACHE:
        return _CACHE['nc']
    nc = bacc.Bacc("TRN2", target_bir_lowering=False, debug=False,
                   enable_asserts=True, num_devices=NCORES)
    xT_d = nc.dram_tensor("xT", [C, T], FR, kind="ExternalInput").ap()
    w_d = nc.dram_tensor("w", [C, 576], FR, kind="ExternalInput").ap()
    b_d = nc.dram_tensor("b", [128, 5], F32, kind="ExternalInput").ap()
    out_d = nc.dram_tensor("out", [T, HPC * HD], F32, kind="ExternalOutput").ap()

    # head slot map: (q_tile, q_lo, k_tile, k_lo, v_tile, v_lo)
    # tile0=[Q0;Q1] tile1=[K0;K1] tile2=[Q2;V0] tile3=[K2;V1] tile4=[V2]
    SLOT = [
        (0, 0, 1, 0, 2, 64),
        (0, 64, 1, 64, 3, 64),
        (2, 0, 3, 0, 4, 0),
    ]

    with tile.TileContext(nc) as tc:
        with (
            tc.tile_pool(name="const", bufs=1) as cpool,
            tc.tile_pool(name="persist", bufs=1) as sb,
        ):
            # ---------- constants ----------
            identr_f = cpool.tile([128, 64], F32)
            nc.gpsimd.memset(identr_f[:], 0.0)
            nc.gpsimd.affine_select(out=identr_f[:], in_=identr_f[:],
                                    compare_op=AL.not_equal, fill=1.0, base=0,
                                    channel_multiplier=1, pattern=[[-1, 64]])
            ident_hi_f = cpool.tile([128, 64], F32)
            nc.gpsimd.memset(ident_hi_f[:], 0.0)
            nc.gpsimd.affine_select(out=ident_hi_f[:], in_=ident_hi_f[:],
                                    compare_op=AL.not_equal, fill=1.0, base=-64,
                                    channel_multiplier=1, pattern=[[-1, 64]])
            identr = cpool.tile([128, 64], FR)
            nc.vector.tensor_copy(identr[:], identr_f[:])
            ident_hi = cpool.tile([128, 64], FR)
            nc.vector.tensor_copy(ident_hi[:], ident_hi_f[:])
            ident65 = cpool.tile([65, 65], F32)
            nc.gpsimd.memset(ident65[:], 0.0)
            nc.gpsimd.affine_select(out=ident65[:], in_=ident65[:],
                                    compare_op=AL.not_equal, fill=1.0, base=0,
                                    channel_multiplier=1, pattern=[[-1, 65]])
            ones_f = cpool.tile([128, 1], F32)
            nc.vector.memset(ones_f[:], 1.0)
            # 4 causal masks [128,512]: keep where f - p - 128*d >= 0 else NEG
            masks = cpool.tile([128, 4 * 512], F32)
            nc.gpsimd.memset(masks[:], 0.0)
            for d in range(4):
                nc.gpsimd.affine_select(
                    out=masks[:, d * 512:(d + 1) * 512],
                    in_=masks[:, d * 512:(d + 1) * 512],
                    compare_op=AL.is_ge, fill=NEG,
                    base=-128 * d, channel_multiplier=-1, pattern=[[1, 512]])
            bias_sb = cpool.tile([128, 5], F32)
            nc.sync.dma_start(bias_sb[:], b_d[:])

            # persistent: projection outputs + V_aug
            qkv_sb = [sb.tile([128, T], FR, name=f"qkv{m}") for m in range(4)]
            qkv_sb.append(sb.tile([64, T], FR, name="qkv4"))
            v_aug = [sb.tile([128, NKC * 65], FR, name=f"vaug{h}")
                     for h in range(HPC)]

            # ---------- phase 1: projection ----------
            with (
                tc.tile_pool(name="wsb", bufs=1) as wpool,
                tc.tile_pool(name="xn", bufs=12) as xpool,
                tc.tile_pool(name="pj", bufs=2, space="PSUM") as pjp,
            ):
                w_sb = [wpool.tile([128, 576], FR, name=f"w{k}") for k in range(6)]
                for k in range(6):
                    nc.sync.dma_start(w_sb[k][:], w_d[128 * k:128 * (k + 1), :])
                for n in range(NQ):
                    xn = []
                    for k in range(6):
                        t = xpool.tile([128, 512], FR, tag="xn", name=f"xn{n}_{k}")
                        nc.sync.dma_start(
                            t[:], xT_d[128 * k:128 * (k + 1), 512 * n:512 * (n + 1)])
                        xn.append(t)
                    for m in range(5):
                        mw = 128 if m < 4 else 64
                        pj = pjp.tile([128, 512], F32, tag="pj", name=f"pj{n}_{m}")
                        for k in range(6):
                            nc.tensor.matmul(pj[:mw, :],
                                             lhsT=w_sb[k][:, 128 * m:128 * m + mw],
                                             rhs=xn[k][:],
                                             start=(k == 0), stop=(k == 5))
                        nc.vector.tensor_scalar(
                            out=qkv_sb[m][:mw, 512 * n:512 * (n + 1)],
                            in0=pj[:mw, :], scalar1=bias_sb[:mw, m:m + 1],
                            scalar2=None, op0=AL.add)

            # ---------- phase 1.5: V transposes into v_aug ----------
            with tc.tile_pool(name="vtr", bufs=4, space="PSUM") as vtp:
                for h in range(HPC):
                    vt, vlo = SLOT[h][4], SLOT[h][5]
                    idn = identr if vlo == 0 else ident_hi
                    for i in range(NKC):
                        pt = vtp.tile([128, 64], FR, tag="vt", name=f"vt{h}_{i}")
                        nc.tensor.transpose(
                            pt[:], qkv_sb[vt][vlo:vlo + 64, 128 * i:128 * (i + 1)],
                            idn[vlo:vlo + 64, :])
                        nc.vector.tensor_copy(v_aug[h][:, 65 * i:65 * i + 64], pt[:])
                        nc.vector.tensor_copy(
                            v_aug[h][:, 65 * i + 64:65 * i + 65], ones_f[:])

            # ---------- phase 2: attention ----------
            with (
                tc.tile_pool(name="ps_s", bufs=2, space="PSUM") as psp,
                tc.tile_pool(name="po", bufs=1, space="PSUM") as pop,
                tc.tile_pool(name="pT", bufs=3) as ptp,
                tc.tile_pool(name="oT", bufs=2) as otp,
                tc.tile_pool(name="outt", bufs=4) as outp,
                tc.tile_pool(name="rcp", bufs=4) as rcpool,
            ):
                for h in range(HPC):
                    qt, qlo, kt, klo = SLOT[h][0], SLOT[h][1], SLOT[h][2], SLOT[h][3]
                    for J in range(NQ):
                        nK = 4 * (J + 1)
                        po = pop.tile([65, 512], F32, tag="po", name=f"po{h}_{J}")
                        qs = qkv_sb[qt][qlo:qlo + 64, 512 * J:512 * (J + 1)]
                        for g0 in range(0, nK, GRP):
                            g1 = min(g0 + GRP, nK)
                            wid = 512 * (g1 - g0)
                            ps_s = psp.tile([128, GRP * 512], F32, tag="ps_s",
                                            name=f"ps{h}_{J}_{g0}")
                            for kc in range(g0, g1):
                                sl = slice(512 * (kc - g0), 512 * (kc - g0 + 1))
                                nc.tensor.matmul(
                                    ps_s[:, sl],
                                    lhsT=qkv_sb[kt][klo:klo + 64,
                                                    128 * kc:128 * (kc + 1)],
                                    rhs=qs, start=True, stop=True)
                                d = kc - 4 * J
                                if d >= 0:
                                    nc.vector.tensor_tensor(
                                        out=ps_s[:, sl], in0=ps_s[:, sl],
                                        in1=masks[:, 512 * d:512 * (d + 1)],
                                        op=AL.add)
                            pT = ptp.tile([128, GRP * 512], FR, tag="pT",
                                          name=f"pT{h}_{J}_{g0}")
                            nc.scalar.activation(pT[:, :wid], ps_s[:, :wid],
                                                 AF.Exp, scale=0.125)
                            for kc in range(g0, g1):
                                sl = slice(512 * (kc - g0), 512 * (kc - g0 + 1))
                                nc.tensor.matmul(
                                    po[:], lhsT=v_aug[h][:, 65 * kc:65 * kc + 65],
                                    rhs=pT[:, sl],
                                    start=(kc == 0), stop=(kc == nK - 1))
                        oT = otp.tile([65, 512], F32, tag="oT", name=f"oT{h}_{J}")
                        nc.vector.tensor_copy(oT[:], po[:])
                        for i in range(4):
                            ptr = pop.tile([128, 65], F32, tag="potr",
                                           name=f"ptr{h}_{J}_{i}")
                            nc.tensor.transpose(ptr[:], oT[:, 128 * i:128 * (i + 1)],
                                                ident65[:])
                            r = rcpool.tile([128, 1], F32, tag="rcp",
                                            name=f"r{h}_{J}_{i}")
                            nc.vector.reciprocal(r[:], ptr[:, 64:65])
                            ot = outp.tile([128, HD], F32, tag="outt",
                                           name=f"ot{h}_{J}_{i}")
                            nc.vector.tensor_scalar(out=ot[:], in0=ptr[:, 0:64],
                                                    scalar1=r[:], scalar2=None,
                                                    op0=AL.mult)
                            nc.sync.dma_start(
                                out_d[512 * J + 128 * i:512 * J + 128 * (i + 1),
                                      HD * h:HD * (h + 1)], ot[:])

    nc.compile()
    _CACHE['nc'] = nc
    return nc


def _prep_inputs(x, w_qkv, b_qkv):
    """Host-side sharding: per-core xT, column-reordered weight stack, bias."""
    x = np.asarray(x, dtype=np.float32)
    w_qkv = np.asarray(w_qkv, dtype=np.float32)
    b_qkv = np.asarray(b_qkv, dtype=np.float32)
    xTs = [np.ascontiguousarray(x[b].T) for b in range(B)]
    in_maps = []
    for c in range(NCORES):
        b_idx, g = c // 4, c % 4
        H = [3 * g, 3 * g + 1, 3 * g + 2]
        q = lambda h: np.arange(64 * h, 64 * (h + 1))
        k = lambda h: np.arange(C + 64 * h, C + 64 * (h + 1))
        v = lambda h: np.arange(2 * C + 64 * h, 2 * C + 64 * (h + 1))
        cols = np.concatenate([
            q(H[0]), q(H[1]),      # tile0
            k(H[0]), k(H[1]),      # tile1
            q(H[2]), v(H[0]),      # tile2
            k(H[2]), v(H[1]),      # tile3
            v(H[2]),               # tile4 (64)
        ])
        w_stack = np.ascontiguousarray(w_qkv[:, cols])
        b_stack = b_qkv[cols]
        bias_pad = np.zeros((128, 5), dtype=np.float32)
        for m in range(4):
            bias_pad[:, m] = b_stack[128 * m:128 * (m + 1)]
        bias_pad[:64, 4] = b_stack[512:576]
        in_maps.append({"xT": xTs[b_idx], "w": w_stack, "b": bias_pad})
    return in_maps


def _run(x, w_qkv, b_qkv, n_head, **run_kwargs):
    assert int(n_head) == NH and x.shape == (B, T, C)
    nc = _build()
    in_maps = _prep_inputs(x, w_qkv, b_qkv)
    res = bass_utils.run_bass_kernel_spmd(
        nc, in_maps, core_ids=list(range(NCORES)), **run_kwargs)
    out = np.empty((B, T, 3 * NH * HD // 3), dtype=np.float32)
    out = np.empty((B, T, C), dtype=np.float32)
    for c in range(NCORES):
        b_idx, g = c // 4, c % 4
        out[b_idx, :, 192 * g:192 * (g + 1)] = res.results[c]["out"]
    return out, res


def kernel(x, w_qkv, b_qkv, n_head):
    return _run(x, w_qkv, b_qkv, n_head)[0]


# revision 2
# speedup vs baseline: 1.1088x; 1.1088x over previous
"""Trainium2 Bass kernel for fused causal multi-head attention
(qkv projection + causal softmax attention), B=2, T=4096, C=768, nH=12.

Sharding: 8 cores, core c -> batch b=c//4, head group g=c%4 (3 heads each).
Each core computes qkv projection for its 3 heads from x[b] (host-transposed
to xT [C,T]) with a column-sharded, column-reordered weight stack, then
attention in the S^T orientation:
  S^T[k,q] = K Q^T  (lhsT = K^T slices, rhs = Q^T)  -> psum
  P^T = exp(S^T/8)  (scalar engine, psum -> sbuf fp32r)
  O^T = [V|1]^T P^T (ones column gives the softmax denominator)
  O   = transpose(O^T) / denom   -> DMA out
All matmuls run in float32r (TF32-class, 4x the fp32 rate).
"""
import sys
sys.path.insert(0, '/opt/trn_rl_repo')
import numpy as np

import concourse.bass as bass
import concourse.tile as tile
from concourse import bacc, mybir
from concourse import bass_utils

B, T, C, NH = 2, 4096, 768, 12
HD = 64
HPC = 3            # heads per core
NCORES = 8
NQ = T // 512      # q-chunks of 512
NKC = T // 128     # k-chunks of 128
GRP = 3            # k-chunks per exp group (psum [128, 1536] x2 bufs)
NEG = -1.0e35

FR = mybir.dt.float32r
BF = mybir.dt.bfloat16
USE_BF16 = True
CD = BF if USE_BF16 else FR
F32 = mybir.dt.float32
AF = mybir.ActivationFunctionType
AL = mybir.AluOpType

_CACHE = {}


def _build():
    if 'nc' in _CACHE:
        return _CACHE['nc']
    nc = bacc.Bacc("TRN2", target_bir_lowering=False, debug=False,
                   enable_asserts=True, num_devices=NCORES)
    xT_d = nc.dram_tensor("xT", [C, T], CD, kind="ExternalInput").ap()
    w_d = nc.dram_tensor("w", [C, 576], CD, kind="ExternalInput").ap()
    b_d = nc.dram_tensor("b", [128, 5], F32, kind="ExternalInput").ap()
    out_d = nc.dram_tensor("out", [T, HPC * HD], F32, kind="ExternalOutput").ap()

    # head slot map: (q_tile, q_lo, k_tile, k_lo, v_tile, v_lo)
    # tile0=[Q0;Q1] tile1=[K0;K1] tile2=[Q2;V0] tile3=[K2;V1] tile4=[V2]
    SLOT = [
        (0, 0, 1, 0, 2, 64),
        (0, 64, 1, 64, 3, 64),
        (2, 0, 3, 0, 4, 0),
    ]

    with tile.TileContext(nc) as tc:
        with (
            tc.tile_pool(name="const", bufs=1) as cpool,
            tc.tile_pool(name="persist", bufs=1) as sb,
        ):
            # ---------- constants ----------
            identr_f = cpool.tile([128, 64], F32)
            nc.gpsimd.memset(identr_f[:], 0.0)
            nc.gpsimd.affine_select(out=identr_f[:], in_=identr_f[:],
                                    compare_op=AL.not_equal, fill=1.0, base=0,
                                    channel_multiplier=1, pattern=[[-1, 64]])
            ident_hi_f = cpool.tile([128, 64], F32)
            nc.gpsimd.memset(ident_hi_f[:], 0.0)
            nc.gpsimd.affine_select(out=ident_hi_f[:], in_=ident_hi_f[:],
                                    compare_op=AL.not_equal, fill=1.0, base=-64,
                                    channel_multiplier=1, pattern=[[-1, 64]])
            identr = cpool.tile([128, 64], CD)
            nc.vector.tensor_copy(identr[:], identr_f[:])
            ident_hi = cpool.tile([128, 64], CD)
            nc.vector.tensor_copy(ident_hi[:], ident_hi_f[:])
            ident65 = cpool.tile([65, 65], F32)
            nc.gpsimd.memset(ident65[:], 0.0)
            nc.gpsimd.affine_select(out=ident65[:], in_=ident65[:],
                                    compare_op=AL.not_equal, fill=1.0, base=0,
                                    channel_multiplier=1, pattern=[[-1, 65]])
            ones_f = cpool.tile([128, 1], F32)
            nc.vector.memset(ones_f[:], 1.0)
            # 4 causal masks [128,512]: keep where f - p - 128*d >= 0 else NEG
            masks = cpool.tile([128, 4 * 512], F32)
            nc.gpsimd.memset(masks[:], 0.0)
            for d in range(4):
                nc.gpsimd.affine_select(
                    out=masks[:, d * 512:(d + 1) * 512],
                    in_=masks[:, d * 512:(d + 1) * 512],
                    compare_op=AL.is_ge, fill=NEG,
                    base=-128 * d, channel_multiplier=-1, pattern=[[1, 512]])
            bias_sb = cpool.tile([128, 5], F32)
            nc.sync.dma_start(bias_sb[:], b_d[:])

            # persistent: projection outputs + V_aug
            qkv_sb = [sb.tile([128, T], CD, name=f"qkv{m}") for m in range(4)]
            qkv_sb.append(sb.tile([64, T], CD, name="qkv4"))
            v_aug = [sb.tile([128, NKC * 65], CD, name=f"vaug{h}")
                     for h in range(HPC)]

            # ---------- phase 1: projection ----------
            with (
                tc.tile_pool(name="wsb", bufs=1) as wpool,
                tc.tile_pool(name="xn", bufs=12) as xpool,
                tc.tile_pool(name="pj", bufs=2, space="PSUM") as pjp,
            ):
                w_sb = [wpool.tile([128, 576], CD, name=f"w{k}") for k in range(6)]
                for k in range(6):
                    nc.sync.dma_start(w_sb[k][:], w_d[128 * k:128 * (k + 1), :])
                for n in range(NQ):
                    xn = []
                    for k in range(6):
                        t = xpool.tile([128, 512], CD, tag="xn", name=f"xn{n}_{k}")
                        nc.sync.dma_start(
                            t[:], xT_d[128 * k:128 * (k + 1), 512 * n:512 * (n + 1)])
                        xn.append(t)
                    for m in range(5):
                        mw = 128 if m < 4 else 64
                        pj = pjp.tile([128, 512], F32, tag="pj", name=f"pj{n}_{m}")
                        for k in range(6):
                            nc.tensor.matmul(pj[:mw, :],
                                             lhsT=w_sb[k][:, 128 * m:128 * m + mw],
                                             rhs=xn[k][:],
                                             start=(k == 0), stop=(k == 5))
                        nc.vector.tensor_scalar(
                            out=qkv_sb[m][:mw, 512 * n:512 * (n + 1)],
                            in0=pj[:mw, :], scalar1=bias_sb[:mw, m:m + 1],
                            scalar2=None, op0=AL.add)

            # ---------- phase 1.5: V transposes into v_aug ----------
            with tc.tile_pool(name="vtr", bufs=4, space="PSUM") as vtp:
                for h in range(HPC):
                    vt, vlo = SLOT[h][4], SLOT[h][5]
                    idn = identr if vlo == 0 else ident_hi
                    for i in range(NKC):
                        pt = vtp.tile([128, 64], CD, tag="vt", name=f"vt{h}_{i}")
                        nc.tensor.transpose(
                            pt[:], qkv_sb[vt][vlo:vlo + 64, 128 * i:128 * (i + 1)],
                            idn[vlo:vlo + 64, :])
                        nc.vector.tensor_copy(v_aug[h][:, 65 * i:65 * i + 64], pt[:])
                        nc.vector.tensor_copy(
                            v_aug[h][:, 65 * i + 64:65 * i + 65], ones_f[:])

            # ---------- phase 2: attention ----------
            with (
                tc.tile_pool(name="ps_s", bufs=2, space="PSUM") as psp,
                tc.tile_pool(name="po", bufs=1, space="PSUM") as pop,
                tc.tile_pool(name="pT", bufs=3) as ptp,
                tc.tile_pool(name="oT", bufs=2) as otp,
                tc.tile_pool(name="outt", bufs=4) as outp,
                tc.tile_pool(name="rcp", bufs=4) as rcpool,
            ):
                for h in range(HPC):
                    qt, qlo, kt, klo = SLOT[h][0], SLOT[h][1], SLOT[h][2], SLOT[h][3]
                    for J in range(NQ):
                        nK = 4 * (J + 1)
                        po = pop.tile([65, 512], F32, tag="po", name=f"po{h}_{J}")
                        qs = qkv_sb[qt][qlo:qlo + 64, 512 * J:512 * (J + 1)]
                        pending = None  # (pT, g0, g1) awaiting PV
                        for g0 in range(0, nK, GRP):
                            g1 = min(g0 + GRP, nK)
                            wid = 512 * (g1 - g0)
                            ps_s = psp.tile([128, GRP * 512], F32, tag="ps_s",
                                            name=f"ps{h}_{J}_{g0}")
                            for kc in range(g0, g1):
                                sl = slice(512 * (kc - g0), 512 * (kc - g0 + 1))
                                nc.tensor.matmul(
                                    ps_s[:, sl],
                                    lhsT=qkv_sb[kt][klo:klo + 64,
                                                    128 * kc:128 * (kc + 1)],
                                    rhs=qs, start=True, stop=True)
                                d = kc - 4 * J
                                if d >= 0:
                                    nc.vector.tensor_tensor(
                                        out=ps_s[:, sl], in0=ps_s[:, sl],
                                        in1=masks[:, 512 * d:512 * (d + 1)],
                                        op=AL.add)
                            pT = ptp.tile([128, GRP * 512], CD, tag="pT",
                                          name=f"pT{h}_{J}_{g0}")
                            nc.scalar.activation(pT[:, :wid], ps_s[:, :wid],
                                                 AF.Exp, scale=0.125)
                            if pending is not None:
                                pg0, pg1, ppT = pending
                                for kc in range(pg0, pg1):
                                    sl = slice(512 * (kc - pg0), 512 * (kc - pg0 + 1))
                                    nc.tensor.matmul(
                                        po[:], lhsT=v_aug[h][:, 65 * kc:65 * kc + 65],
                                        rhs=ppT[:, sl],
                                        start=(kc == 0), stop=False)
                            pending = (g0, g1, pT)
                        pg0, pg1, ppT = pending
                        for kc in range(pg0, pg1):
                            sl = slice(512 * (kc - pg0), 512 * (kc - pg0 + 1))
                            nc.tensor.matmul(
                                po[:], lhsT=v_aug[h][:, 65 * kc:65 * kc + 65],
                                rhs=ppT[:, sl],
                                start=(kc == 0), stop=(kc == nK - 1))
                        oT = otp.tile([65, 512], F32, tag="oT", name=f"oT{h}_{J}")
                        nc.vector.tensor_copy(oT[:], po[:])
                        for i in range(4):
                            ptr = pop.tile([128, 65], F32, tag="potr",
                                           name=f"ptr{h}_{J}_{i}")
                            nc.tensor.transpose(ptr[:], oT[:, 128 * i:128 * (i + 1)],
                                                ident65[:])
                            r = rcpool.tile([128, 1], F32, tag="rcp",
                                            name=f"r{h}_{J}_{i}")
                            nc.vector.reciprocal(r[:], ptr[:, 64:65])
                            ot = outp.tile([128, HD], F32, tag="outt",
                                           name=f"ot{h}_{J}_{i}")
                            nc.vector.tensor_scalar(out=ot[:], in0=ptr[:, 0:64],
                                                    scalar1=r[:], scalar2=None,
                                                    op0=AL.mult)
                            nc.sync.dma_start(
                                out_d[512 * J + 128 * i:512 * J + 128 * (i + 1),
                                      HD * h:HD * (h + 1)], ot[:])

    nc.compile()
    _CACHE['nc'] = nc
    return nc


def _prep_inputs(x, w_qkv, b_qkv):
    """Host-side sharding: per-core xT, column-reordered weight stack, bias."""
    import ml_dtypes
    cdt = ml_dtypes.bfloat16 if USE_BF16 else np.float32
    x = np.asarray(x, dtype=np.float32)
    w_qkv = np.asarray(w_qkv, dtype=np.float32)
    b_qkv = np.asarray(b_qkv, dtype=np.float32)
    xTs = [np.ascontiguousarray(x[b].T).astype(cdt) for b in range(B)]
    in_maps = []
    for c in range(NCORES):
        b_idx, g = c // 4, c % 4
        H = [3 * g, 3 * g + 1, 3 * g + 2]
        q = lambda h: np.arange(64 * h, 64 * (h + 1))
        k = lambda h: np.arange(C + 64 * h, C + 64 * (h + 1))
        v = lambda h: np.arange(2 * C + 64 * h, 2 * C + 64 * (h + 1))
        cols = np.concatenate([
            q(H[0]), q(H[1]),      # tile0
            k(H[0]), k(H[1]),      # tile1
            q(H[2]), v(H[0]),      # tile2
            k(H[2]), v(H[1]),      # tile3
            v(H[2]),               # tile4 (64)
        ])
        w_stack = np.ascontiguousarray(w_qkv[:, cols]).astype(cdt)
        b_stack = b_qkv[cols]
        bias_pad = np.zeros((128, 5), dtype=np.float32)
        for m in range(4):
            bias_pad[:, m] = b_stack[128 * m:128 * (m + 1)]
        bias_pad[:64, 4] = b_stack[512:576]
        in_maps.append({"xT": xTs[b_idx], "w": w_stack, "b": bias_pad})
    return in_maps


def _run(x, w_qkv, b_qkv, n_head, **run_kwargs):
    assert int(n_head) == NH and x.shape == (B, T, C)
    nc = _build()
    in_maps = _prep_inputs(x, w_qkv, b_qkv)
    res = bass_utils.run_bass_kernel_spmd(
        nc, in_maps, core_ids=list(range(NCORES)), **run_kwargs)
    out = np.empty((B, T, 3 * NH * HD // 3), dtype=np.float32)
    out = np.empty((B, T, C), dtype=np.float32)
    for c in range(NCORES):
        b_idx, g = c // 4, c % 4
        out[b_idx, :, 192 * g:192 * (g + 1)] = res.results[c]["out"]
    return out, res


def kernel(x, w_qkv, b_qkv, n_head):
    return _run(x, w_qkv, b_qkv, n_head)[0]
